# revision 18
# baseline (speedup 1.0000x reference)
"""InfoNCE loss kernel for 8 Trainium2 NeuronCores (symmetric-triangle version).

Math (reference): z = concat(z1, z2) [2N, D] row-normalized; sim = z@z.T/TEMP;
self-diagonal masked; loss = mean(-pos + logsumexp(sim, axis=1)).

sim is SYMMETRIC, so only a triangle of the 16x16 grid of 512-wide band
blocks is computed: 136 blocks instead of 256. Core c (with per-core band
rotation slot s -> band (c+s)%16) computes the canonical pattern
  lhs slot 0:  rhs slots 0..8   (slot 0 = self-diagonal block)
  lhs slot 8:  rhs slots 8..15  (slot 8 = self-diagonal block)
which covers every unordered band pair exactly once across the 8 cores.
Each off-diagonal block contributes exp row-sums (fused scalar-engine
accum_out) to its lhs band AND exp column-sums (fp8e5m2 DoubleRow
ones-matmul over the partition axis) to its rhs band.

Diagonal blocks are THEMSELVES symmetric, so only their upper triangle is
computed: the 128-row chain mm covers columns [128*mm, 512) (N = 512/384/
256/128). Chains 0..2 get -1e6 added onto the raw PSUM self-diagonal
sub-block and a plain exp (no accumulator); the fp8e5 exp tiles are DMAd to
the host, which computes both the upper row sums and (by symmetry) the
lower-triangle row sums from them. The mm=3 chain skips exp entirely: its
raw [128,128] PSUM sub-block is DMAd out and the host does mask+exp+sum.
This keeps the scalar engine off the critical path at both the start (diag0
is the first block) and the end (diag1 is the last).

Processing order puts the odd single-block group [15] mid-row so the tail
is dual pairs followed by the diag1 triangle, whose pending column-sum
matmuls flush between its shrinking chains.

Tricks kept: z pre-scaled by 8 before the e4m3 cast (1/64 folded into the
exp scale); fp8 DoubleRow 256-deep contraction; column-sum matmul emission
deferred past the next group's first chains so the PE never stalls waiting
on the scalar engine's exp. The warm-up burst is sized to bridge the z
slot-0 DMA wait so the PE_HAM clock gate is released before the first real
matmul issues.
"""

from contextlib import ExitStack

import ml_dtypes
import numpy as np

import concourse.bass as bass
import concourse.tile as tile
from concourse import bacc, mybir
from concourse.bass_utils import run_bass_kernel_spmd

N_CORES = 8
N, D = 4096, 1024
ROWS = 2 * N               # 8192 rows of z
NB = 16                    # 512-row bands
BAND = ROWS // NB          # 512
KT = D // 128              # 8 contraction slices (4 DoubleRow pairs)
TEMP = 0.07
INV_T = 1.0 / TEMP
FP8_SCALE = 8.0            # host pre-scale before e4m3 cast
MM_SCALE = INV_T / (FP8_SCALE * FP8_SCALE)
N_WARM = 80                # dummy MMs bridging the slot-0 DMA wait

_CACHE = {}


def _build_graph():
    nc = bacc.Bacc("TRN2", target_bir_lowering=False, debug=False, num_devices=N_CORES)
    z = nc.declare_dram_parameter("z", [NB, 128, KT, BAND], mybir.dt.float8e4, isOutput=False)
    rowacc_d = nc.declare_dram_parameter("rowacc", [128, 2, 4, 4], mybir.dt.float32, isOutput=True)
    colsum_d = nc.declare_dram_parameter("colsum", [1, 15, BAND], mybir.dt.float32, isOutput=True)
    exdiag_d = nc.declare_dram_parameter("exdiag", [128, 2, BAND], mybir.dt.float8e5, isOutput=True)
    psc_d = nc.declare_dram_parameter("psc", [128, 2, 768], mybir.dt.float32, isOutput=True)

    fp32 = mybir.dt.float32
    fp8e4 = mybir.dt.float8e4
    fp8e5 = mybir.dt.float8e5
    AF = mybir.ActivationFunctionType
    DR = mybir.MatmulPerfMode.DoubleRow

    with tile.TileContext(nc) as tc, ExitStack() as ctx:
        zpool = ctx.enter_context(tc.tile_pool(name="z", bufs=1))
        consts = ctx.enter_context(tc.tile_pool(name="consts", bufs=1))
        pspool = ctx.enter_context(tc.tile_pool(name="ps", bufs=3, space="PSUM"))
        cspool = ctx.enter_context(tc.tile_pool(name="cs", bufs=2, space="PSUM"))
        expool = ctx.enter_context(tc.tile_pool(name="ex", bufs=3))
        exfpool = ctx.enter_context(tc.tile_pool(name="exf", bufs=6))
        outpool = ctx.enter_context(tc.tile_pool(name="outp", bufs=1))

        # stage z into SBUF: one whole-band dma_start per slot in slot order
        # (slot 0 is the first block's only dependency)
        zc = []
        for s in range(NB):
            t = zpool.tile([128, KT, BAND], fp8e4, tag=f"zc{s}", name=f"zc{s}")
            nc.sync.dma_start(out=t[:], in_=z[s])
            zc.append(t)

        # warm-up burst: dummy matmuls keep the PE busy from the framework
        # preamble until slot 0 lands, so PE_HAM releases the clock gate
        # (1.2 -> 2.4 GHz) before the real matmul stream starts
        warm = consts.tile([128, 64], fp8e4, tag="warm")
        nc.gpsimd.memset(warm[:], 0.0)
        warmps = cspool.tile([128, BAND], fp32, tag="cs", name="warmps")
        for _ in range(N_WARM):
            nc.tensor.matmul(warmps[0:64, 0:64], lhsT=warm[:], rhs=warm[:],
                             start=True, stop=True)

        # constants: -1e6 * identity (pre-exp self mask, added into the raw
        # PSUM diagonal so exp flushes it to 0) and a fp8 ones block for the
        # DoubleRow column-sum matmuls
        negeye = consts.tile([128, 128], fp32, tag="negeye")
        nc.gpsimd.memset(negeye[:], -1.0e6)
        nc.gpsimd.affine_select(
            out=negeye[:],
            in_=negeye[:],
            compare_op=mybir.AluOpType.is_equal,
            fill=0.0,
            base=0,
            pattern=[[-1, 128]],
            channel_multiplier=1,
        )
        ones8 = consts.tile([128, 2, 128], fp8e5, tag="ones8")
        nc.gpsimd.memset(ones8[:], 1.0)

        rowacc = outpool.tile([128, 2, 4, 4], fp32, tag="rowacc")
        psc_sb = outpool.tile([128, 2, 768], fp32, tag="psc_sb")
        cs_sb = outpool.tile([128, 15, BAND], fp32, tag="cs_sb")
        exdiag = outpool.tile([128, 2, BAND], fp8e5, tag="exdiag")

        def mm_chain(ps_slice, L, s, mm, off=0):
            # [128 rows, 512-off cols] block tile: 4 DoubleRow matmuls, K=1024
            for kp in range(4):
                nc.tensor.matmul(
                    ps_slice,
                    lhsT=zc[L][:, 2 * kp : 2 * kp + 2, 128 * mm : 128 * mm + 128],
                    rhs=zc[s][:, 2 * kp : 2 * kp + 2, off:],
                    start=(kp == 0),
                    stop=(kp == 3),
                    perf_mode=DR,
                )

        # deferred column-sum emission: tensor-engine instructions execute in
        # program order, so the ones-matmuls (which wait on the scalar engine's
        # exp) are emitted after the NEXT group's first chains to avoid PE
        # stalls
        pending_cs = []
        pending_copies = []
        cs_state = {"idx": 0, "tail": False}

        def cs_copy(dst, src_):
            # mid-stream: idle DVE. In the tail the copies are deferred and
            # later emitted on the scalar queue AFTER the final diag exps,
            # so neither the DVE (negeye adds, PSUM staging) nor the exp
            # chain ever waits behind them.
            if cs_state["tail"]:
                if len(pending_copies) % 2 == 0:
                    pending_copies.append(lambda: nc.scalar.copy(dst, src_))
                else:
                    pending_copies.append(lambda: nc.vector.tensor_copy(dst, src_))
            else:
                nc.vector.tensor_copy(dst, src_)

        def flush_cs():
            for fn in pending_cs:
                fn()
            pending_cs.clear()

        def flush_one():
            if pending_cs:
                pending_cs.pop(0)()

        def emit_cs(exq, h):
            # DoubleRow ones-matmul along the partition axis: every output row
            # equals the column sums of the block's 512 rows; row 0 is kept
            ci = cs_state["idx"]
            cs_state["idx"] += 1
            cur = cspool.tile([128, BAND], fp32, tag="cs", name="cs")
            nc.tensor.matmul(
                cur[:], lhsT=ones8[:], rhs=exq[:, 0:2, h, :],
                start=True, stop=False, perf_mode=DR,
            )
            nc.tensor.matmul(
                cur[:], lhsT=ones8[:], rhs=exq[:, 2:4, h, :],
                start=False, stop=True, perf_mode=DR,
            )
            cs_copy(cs_sb[0:1, ci, :], cur[0:1, :])

        def emit_cs_folded(exf):
            # the mm fold already happened on the (otherwise idle) gpsimd,
            # so one DoubleRow ones-matmul finishes the column sums
            ci = cs_state["idx"]
            cs_state["idx"] += 1
            cur = cspool.tile([128, BAND], fp32, tag="cs", name="cs")
            nc.tensor.matmul(
                cur[:], lhsT=ones8[:], rhs=exf[:],
                start=True, stop=True, perf_mode=DR,
            )
            cs_copy(cs_sb[0:1, ci, :], cur[0:1, :])

        half_state = {}

        def emit_cs_half(exq, part):
            # single-block column sums split so the first DR half can fire as
            # soon as the mp0 exps are done (the block has h=0 only)
            if part == 0:
                ci = cs_state["idx"]
                cs_state["idx"] += 1
                cur = cspool.tile([128, BAND], fp32, tag="cs", name="cs")
                half_state["cur"] = cur
                half_state["ci"] = ci
                nc.tensor.matmul(
                    cur[:], lhsT=ones8[:], rhs=exq[:, 0:2, 0, :],
                    start=True, stop=False, perf_mode=DR,
                )
            else:
                cur = half_state["cur"]
                nc.tensor.matmul(
                    cur[:], lhsT=ones8[:], rhs=exq[:, 2:4, 0, :],
                    start=False, stop=True, perf_mode=DR,
                )
                cs_copy(cs_sb[0:1, half_state["ci"], :], cur[0:1, :])

        def do_diag(d, L, last=False):
            # self block (slot L, slot L), upper triangle only: chain mm
            # covers cols [128*mm, 512), packed two chains per PSUM tile.
            # Chains 0,1: -1e6 onto the diag sub-block (local cols 0:128),
            # plain exp into the persistent exdiag strip (no accumulator).
            # Chains 2,3: raw PSUM staged to SBUF and DMAd out; the host
            # does mask+exp+sum (keeps the scalar engine off the critical
            # path at both ends of the kernel).
            for mp in range(2):
                ps = pspool.tile([128, 2, BAND], fp32, tag="ps", name="ps")
                for h in range(2):
                    mm = 2 * mp + h
                    if last:
                        flush_one()
                    w = BAND - 128 * mm
                    mm_chain(ps[:, h, 0:w], L, L, mm, off=128 * mm)
                    if not last and mm == 0:
                        flush_cs()
                    if mm == 0:
                        # negeye + exp right behind the chain that feeds it
                        nc.vector.tensor_add(
                            ps[:, h, 0:128], ps[:, h, 0:128], negeye[:]
                        )
                        nc.scalar.activation(
                            out=exdiag[:, d, :], in_=ps[:, h, :],
                            func=AF.Exp, bias=0.0, scale=MM_SCALE,
                        )
                    elif mm == 1:
                        # chains 1..3 go to the host raw; stage off PSUM now
                        nc.vector.tensor_copy(
                            psc_sb[:, d, 0:384], ps[:, h, 0:384]
                        )
                if mp == 1:
                    def stage_psc(d=d, ps=ps):
                        nc.vector.tensor_copy(psc_sb[:, d, 384:640], ps[:, 0, 0:256])
                        nc.vector.tensor_copy(psc_sb[:, d, 640:768], ps[:, 1, 0:128])
                        nc.sync.dma_start(out=psc_d[:, d, :], in_=psc_sb[:, d, :])
                    if last:
                        stage_psc()
                    else:
                        # keep the DVE clear for the upcoming negeye adds;
                        # the copies ride the next group's flush slot
                        pending_cs.append(stage_psc)

        def do_pair(d, L, slot, blocks, fold=True, flush_here=True):
            exq = expool.tile([128, 4, 2, BAND], fp8e5, tag="exq")
            if len(blocks) == 2:
                for mm in range(4):
                    ps = pspool.tile([128, 2, BAND], fp32, tag="ps", name="ps")
                    for h, s_ in enumerate(blocks):
                        mm_chain(ps[:, h, :], L, s_, mm)
                    if mm == 1 and flush_here:
                        flush_cs()
                    # fused exp + row-sum over both blocks' 1024 cols
                    nc.scalar.activation(
                        out=exq[:, mm, :, :], in_=ps[:], func=AF.Exp, bias=0.0,
                        scale=MM_SCALE, accum_out=rowacc[:, d, mm, slot : slot + 1],
                    )
            else:
                for mp in range(2):
                    ps = pspool.tile([128, 2, BAND], fp32, tag="ps", name="ps")
                    for h in range(2):
                        mm_chain(ps[:, h, :], L, blocks[0], 2 * mp + h)
                    if mp == 0:
                        flush_cs()
                    for h in range(2):
                        mm = 2 * mp + h
                        nc.scalar.activation(
                            out=exq[:, mm, 0, :], in_=ps[:, h, :], func=AF.Exp, bias=0.0,
                            scale=MM_SCALE, accum_out=rowacc[:, d, mm, slot : slot + 1],
                        )
            if fold:
                for h in range(len(blocks)):
                    exf = exfpool.tile([128, 2, BAND], fp8e5, tag="exf", name="exf")
                    nc.gpsimd.tensor_add(
                        exf[:], exq[:, 0:2, h, :], exq[:, 2:4, h, :]
                    )
                    pending_cs.append(lambda exf=exf: emit_cs_folded(exf))
            elif len(blocks) == 2:
                for h in range(2):
                    pending_cs.append(lambda exq=exq, h=h: emit_cs(exq, h))
            else:
                pending_cs.append(lambda exq=exq: emit_cs_half(exq, 0))
                pending_cs.append(lambda exq=exq: emit_cs_half(exq, 1))

        do_diag(0, 0)
        do_pair(0, 0, 0, [1, 2])
        do_pair(0, 0, 1, [3, 4])
        do_pair(0, 0, 2, [5, 6])
        do_pair(0, 0, 3, [7, 8])
        # diag0's exp strip can fly as soon as its last exp lands
        nc.sync.dma_start(out=exdiag_d[:, 0:1, :], in_=exdiag[:, 0:1, :])
        do_pair(1, 8, 3, [15], fold=False)
        do_pair(1, 8, 0, [9, 10], flush_here=False)
        do_pair(1, 8, 1, [11, 12])
        do_pair(1, 8, 2, [13, 14], fold=False)
        nc.sync.dma_start(out=rowacc_d[:], in_=rowacc[:])
        cs_state["tail"] = True
        do_diag(1, 8, last=True)
        flush_cs()
        for fn in pending_copies:
            fn()
        pending_copies.clear()
        nc.sync.dma_start(out=exdiag_d[:, 1:2, :], in_=exdiag[:, 1:2, :])
        nc.sync.dma_start(out=colsum_d[:], in_=cs_sb[0:1, :, :])

    nc.compile()
    return nc


def _make_in_maps(z1: np.ndarray, z2: np.ndarray):
    z = np.concatenate([z1, z2], axis=0)          # [8192, 1024] f32
    zt = (z.T * FP8_SCALE).astype(np.float32)     # [1024, 8192]
    # [NB, 128, KT, BAND] band-major fp8 tiles: band, k-within-tile, k-tile, col
    zb = np.ascontiguousarray(
        zt.reshape(KT, 128, NB, BAND).transpose(2, 1, 0, 3)
    ).astype(ml_dtypes.float8_e4m3)
    return [
        {"z": np.ascontiguousarray(zb[[(c + s) % NB for s in range(NB)]])}
        for c in range(N_CORES)
    ]


def kernel(z1: np.ndarray, z2: np.ndarray) -> np.ndarray:
    assert z1.shape == (N, D) and z2.shape == (N, D)
    in_maps = _make_in_maps(z1, z2)

    if "nc" not in _CACHE:
        _CACHE["nc"] = _build_graph()
    res = run_bass_kernel_spmd(_CACHE["nc"], in_maps, core_ids=list(range(N_CORES)))

    S = np.zeros(ROWS, np.float64)
    for c in range(N_CORES):
        r = res.results[c]
        ra = np.asarray(r["rowacc"], dtype=np.float64)     # [128, 2, 4, 4]
        cs = np.asarray(r["colsum"], dtype=np.float64)[0]  # [15, 512]
        ed = np.asarray(r["exdiag"], dtype=np.float64)     # [128, 2, 512]
        pc = np.asarray(r["psc"], dtype=np.float64)        # [128, 2, 768]
        for d, L in ((0, 0), (1, 8)):
            b = (c + L) % NB
            base = BAND * b
            vals = ra[:, d, :, :].sum(axis=2)                 # [128 p, 4 m]
            S[base : base + BAND] += vals.T.reshape(BAND)
            # diag block, upper-triangle chains: chains 0,1 as on-chip exp
            # tiles, chains 2,3 as raw sim/(64T) PSUM (mask+exp here).
            # Upper row sums + (by symmetry) lower row sums = column sums
            # of the strictly-upper regions.
            for mm in range(4):
                w = BAND - 128 * mm
                if mm == 0:
                    E = ed[:, d, 0:w]                         # rows 0..128
                else:
                    off = (0, 0, 384, 640)[mm]
                    E = np.exp(pc[:, d, off : off + w] * MM_SCALE)
                    np.fill_diagonal(E, 0.0)
                S[base + 128 * mm : base + 128 * (mm + 1)] += E.sum(axis=1)
                if w > 128:
                    S[base + 128 * (mm + 1) : base + BAND] += E[:, 128:w].sum(axis=0)
        for ci, s in enumerate(list(range(1, 9)) + list(range(9, 16))):
            b = (c + s) % NB
            S[BAND * b : BAND * (b + 1)] += cs[ci, :]
    pos = (z1.astype(np.float64) * z2.astype(np.float64)).sum(axis=1) / TEMP
    loss = np.log(S).mean() - pos.mean()
    return np.asarray(loss, dtype=np.float32)


# revision 19
# speedup vs baseline: 1.0280x; 1.0280x over previous
"""InfoNCE loss kernel for 8 Trainium2 NeuronCores (symmetric-triangle version).

Math (reference): z = concat(z1, z2) [2N, D] row-normalized; sim = z@z.T/TEMP;
self-diagonal masked; loss = mean(-pos + logsumexp(sim, axis=1)).

sim is SYMMETRIC, so only a triangle of the 16x16 grid of 512-wide band
blocks is computed: 136 blocks instead of 256. Core c (with per-core band
rotation slot s -> band (c+s)%16) computes the canonical pattern
  lhs slot 0:  rhs slots 0..8   (slot 0 = self-diagonal block)
  lhs slot 8:  rhs slots 8..15  (slot 8 = self-diagonal block)
which covers every unordered band pair exactly once across the 8 cores.
Each off-diagonal block contributes exp row-sums (fused scalar-engine
accum_out) to its lhs band AND exp column-sums (fp8e5m2 DoubleRow
ones-matmul over the partition axis) to its rhs band.

Diagonal blocks are THEMSELVES symmetric, so only their upper triangle is
computed: the 128-row chain mm covers columns [128*mm, 512) (N = 512/384/
256/128). Chains 0..2 get -1e6 added onto the raw PSUM self-diagonal
sub-block and a plain exp (no accumulator); the fp8e5 exp tiles are DMAd to
the host, which computes both the upper row sums and (by symmetry) the
lower-triangle row sums from them. The mm=3 chain skips exp entirely: its
raw [128,128] PSUM sub-block is DMAd out and the host does mask+exp+sum.
This keeps the scalar engine off the critical path at both the start (diag0
is the first block) and the end (diag1 is the last).

Processing order puts the odd single-block group [15] mid-row so the tail
is dual pairs followed by the diag1 triangle, whose pending column-sum
matmuls flush between its shrinking chains.

Tricks kept: z pre-scaled by 8 before the e4m3 cast (1/64 folded into the
exp scale); fp8 DoubleRow 256-deep contraction; column-sum matmul emission
deferred past the next group's first chains so the PE never stalls waiting
on the scalar engine's exp. The warm-up burst is sized to bridge the z
slot-0 DMA wait so the PE_HAM clock gate is released before the first real
matmul issues.
"""

from contextlib import ExitStack

import ml_dtypes
import numpy as np

import concourse.bass as bass
import concourse.tile as tile
from concourse import bacc, mybir
from concourse.bass_utils import run_bass_kernel_spmd

N_CORES = 8
N, D = 4096, 1024
ROWS = 2 * N               # 8192 rows of z
NB = 16                    # 512-row bands
BAND = ROWS // NB          # 512
KT = D // 128              # 8 contraction slices (4 DoubleRow pairs)
TEMP = 0.07
INV_T = 1.0 / TEMP
FP8_SCALE = 8.0            # host pre-scale before e4m3 cast
MM_SCALE = INV_T / (FP8_SCALE * FP8_SCALE)
N_WARM = 80                # dummy MMs bridging the slot-0 DMA wait

_CACHE = {}


def _build_graph():
    nc = bacc.Bacc("TRN2", target_bir_lowering=False, debug=False, num_devices=N_CORES)
    z = nc.declare_dram_parameter("z", [NB, 128, KT, BAND], mybir.dt.float8e4, isOutput=False)
    rowacc_d = nc.declare_dram_parameter("rowacc", [128, 2, 4, 4], mybir.dt.float32, isOutput=True)
    colsum_d = nc.declare_dram_parameter("colsum", [1, 15, BAND], mybir.dt.float32, isOutput=True)
    exdiag_d = nc.declare_dram_parameter("exdiag", [128, 2, BAND], mybir.dt.float8e5, isOutput=True)
    psc_d = nc.declare_dram_parameter("psc", [128, 2, 768], mybir.dt.float32, isOutput=True)
    p15_d = nc.declare_dram_parameter("p15", [128, 2, 2, BAND], mybir.dt.float32, isOutput=True)

    fp32 = mybir.dt.float32
    fp8e4 = mybir.dt.float8e4
    fp8e5 = mybir.dt.float8e5
    AF = mybir.ActivationFunctionType
    DR = mybir.MatmulPerfMode.DoubleRow

    with tile.TileContext(nc) as tc, ExitStack() as ctx:
        zpool = ctx.enter_context(tc.tile_pool(name="z", bufs=1))
        consts = ctx.enter_context(tc.tile_pool(name="consts", bufs=1))
        pspool = ctx.enter_context(tc.tile_pool(name="ps", bufs=3, space="PSUM"))
        cspool = ctx.enter_context(tc.tile_pool(name="cs", bufs=2, space="PSUM"))
        expool = ctx.enter_context(tc.tile_pool(name="ex", bufs=3))
        exfpool = ctx.enter_context(tc.tile_pool(name="exf", bufs=6))
        outpool = ctx.enter_context(tc.tile_pool(name="outp", bufs=1))

        # stage z into SBUF: one whole-band dma_start per slot in slot order
        # (slot 0 is the first block's only dependency)
        zc = []
        for s in range(NB):
            t = zpool.tile([128, KT, BAND], fp8e4, tag=f"zc{s}", name=f"zc{s}")
            nc.sync.dma_start(out=t[:], in_=z[s])
            zc.append(t)

        # warm-up burst: dummy matmuls keep the PE busy from the framework
        # preamble until slot 0 lands, so PE_HAM releases the clock gate
        # (1.2 -> 2.4 GHz) before the real matmul stream starts
        warm = consts.tile([128, 64], fp8e4, tag="warm")
        nc.gpsimd.memset(warm[:], 0.0)
        warmps = cspool.tile([128, BAND], fp32, tag="cs", name="warmps")
        for _ in range(N_WARM):
            nc.tensor.matmul(warmps[0:64, 0:64], lhsT=warm[:], rhs=warm[:],
                             start=True, stop=True)

        # constants: -1e6 * identity (pre-exp self mask, added into the raw
        # PSUM diagonal so exp flushes it to 0) and a fp8 ones block for the
        # DoubleRow column-sum matmuls
        negeye = consts.tile([128, 128], fp32, tag="negeye")
        nc.gpsimd.memset(negeye[:], -1.0e6)
        nc.gpsimd.affine_select(
            out=negeye[:],
            in_=negeye[:],
            compare_op=mybir.AluOpType.is_equal,
            fill=0.0,
            base=0,
            pattern=[[-1, 128]],
            channel_multiplier=1,
        )
        ones8 = consts.tile([128, 2, 128], fp8e5, tag="ones8")
        nc.gpsimd.memset(ones8[:], 1.0)

        rowacc = outpool.tile([128, 2, 4, 4], fp32, tag="rowacc")
        psc_sb = outpool.tile([128, 2, 768], fp32, tag="psc_sb")
        p15_sb = outpool.tile([128, 2, 2, BAND], fp32, tag="p15_sb")
        cs_sb = outpool.tile([128, 15, BAND], fp32, tag="cs_sb")
        exdiag = outpool.tile([128, 2, BAND], fp8e5, tag="exdiag")

        def mm_chain(ps_slice, L, s, mm, off=0):
            # [128 rows, 512-off cols] block tile: 4 DoubleRow matmuls, K=1024
            for kp in range(4):
                nc.tensor.matmul(
                    ps_slice,
                    lhsT=zc[L][:, 2 * kp : 2 * kp + 2, 128 * mm : 128 * mm + 128],
                    rhs=zc[s][:, 2 * kp : 2 * kp + 2, off:],
                    start=(kp == 0),
                    stop=(kp == 3),
                    perf_mode=DR,
                )

        # deferred column-sum emission: tensor-engine instructions execute in
        # program order, so the ones-matmuls (which wait on the scalar engine's
        # exp) are emitted after the NEXT group's first chains to avoid PE
        # stalls
        pending_cs = []
        pending_copies = []
        cs_state = {"idx": 0, "tail": False}

        def cs_copy(dst, src_):
            # mid-stream: idle DVE. In the tail the copies are deferred and
            # later emitted on the scalar queue AFTER the final diag exps,
            # so neither the DVE (negeye adds, PSUM staging) nor the exp
            # chain ever waits behind them.
            if cs_state["tail"]:
                if len(pending_copies) % 2 == 0:
                    pending_copies.append(lambda: nc.scalar.copy(dst, src_))
                else:
                    pending_copies.append(lambda: nc.vector.tensor_copy(dst, src_))
            else:
                nc.vector.tensor_copy(dst, src_)

        def flush_cs():
            for fn in pending_cs:
                fn()
            pending_cs.clear()

        def flush_one():
            if pending_cs:
                pending_cs.pop(0)()

        def emit_cs(exq, h):
            # DoubleRow ones-matmul along the partition axis: every output row
            # equals the column sums of the block's 512 rows; row 0 is kept
            ci = cs_state["idx"]
            cs_state["idx"] += 1
            cur = cspool.tile([128, BAND], fp32, tag="cs", name="cs")
            nc.tensor.matmul(
                cur[:], lhsT=ones8[:], rhs=exq[:, 0:2, h, :],
                start=True, stop=False, perf_mode=DR,
            )
            nc.tensor.matmul(
                cur[:], lhsT=ones8[:], rhs=exq[:, 2:4, h, :],
                start=False, stop=True, perf_mode=DR,
            )
            cs_copy(cs_sb[0:1, ci, :], cur[0:1, :])

        def emit_cs_folded(exf):
            # the mm fold already happened on the (otherwise idle) gpsimd,
            # so one DoubleRow ones-matmul finishes the column sums
            ci = cs_state["idx"]
            cs_state["idx"] += 1
            cur = cspool.tile([128, BAND], fp32, tag="cs", name="cs")
            nc.tensor.matmul(
                cur[:], lhsT=ones8[:], rhs=exf[:],
                start=True, stop=True, perf_mode=DR,
            )
            cs_copy(cs_sb[0:1, ci, :], cur[0:1, :])

        half_state = {}

        def emit_cs_half(exq, part):
            # single-block column sums split so the first DR half can fire as
            # soon as the mp0 exps are done (the block has h=0 only)
            if part == 0:
                ci = cs_state["idx"]
                cs_state["idx"] += 1
                cur = cspool.tile([128, BAND], fp32, tag="cs", name="cs")
                half_state["cur"] = cur
                half_state["ci"] = ci
                nc.tensor.matmul(
                    cur[:], lhsT=ones8[:], rhs=exq[:, 0:2, 0, :],
                    start=True, stop=False, perf_mode=DR,
                )
            else:
                cur = half_state["cur"]
                nc.tensor.matmul(
                    cur[:], lhsT=ones8[:], rhs=exq[:, 2:4, 0, :],
                    start=False, stop=True, perf_mode=DR,
                )
                cs_copy(cs_sb[0:1, half_state["ci"], :], cur[0:1, :])

        def do_diag(d, L, last=False):
            # self block (slot L, slot L), upper triangle only: chain mm
            # covers cols [128*mm, 512), packed two chains per PSUM tile.
            # Chains 0,1: -1e6 onto the diag sub-block (local cols 0:128),
            # plain exp into the persistent exdiag strip (no accumulator).
            # Chains 2,3: raw PSUM staged to SBUF and DMAd out; the host
            # does mask+exp+sum (keeps the scalar engine off the critical
            # path at both ends of the kernel).
            for mp in range(2):
                ps = pspool.tile([128, 2, BAND], fp32, tag="ps", name="ps")
                for h in range(2):
                    mm = 2 * mp + h
                    if last:
                        flush_one()
                    w = BAND - 128 * mm
                    mm_chain(ps[:, h, 0:w], L, L, mm, off=128 * mm)
                    if not last and mm == 0:
                        flush_cs()
                    if mm == 0:
                        # negeye + exp right behind the chain that feeds it
                        nc.vector.tensor_add(
                            ps[:, h, 0:128], ps[:, h, 0:128], negeye[:]
                        )
                        nc.scalar.activation(
                            out=exdiag[:, d, :], in_=ps[:, h, :],
                            func=AF.Exp, bias=0.0, scale=MM_SCALE,
                        )
                    elif mm == 1:
                        # chains 1..3 go to the host raw; stage off PSUM now
                        nc.vector.tensor_copy(
                            psc_sb[:, d, 0:384], ps[:, h, 0:384]
                        )
                if mp == 1:
                    def stage_psc(d=d, ps=ps):
                        nc.vector.tensor_copy(psc_sb[:, d, 384:640], ps[:, 0, 0:256])
                        nc.vector.tensor_copy(psc_sb[:, d, 640:768], ps[:, 1, 0:128])
                        nc.sync.dma_start(out=psc_d[:, d, :], in_=psc_sb[:, d, :])
                    if last:
                        stage_psc()
                    else:
                        # keep the DVE clear for the upcoming negeye adds;
                        # the copies ride the next group's flush slot
                        pending_cs.append(stage_psc)

        def do_single_raw(L, s):
            # the odd 15th block: raw sim/(64T) PSUM straight to the host
            # (exp + row/col sums there) — no scalar-engine work at all, so
            # the mid-row exp backlog and its scheduler fallout disappear
            for mp in range(2):
                ps = pspool.tile([128, 2, BAND], fp32, tag="ps", name="ps")
                for h in range(2):
                    mm_chain(ps[:, h, :], L, s, 2 * mp + h)
                if mp == 0:
                    flush_cs()
                nc.vector.tensor_copy(p15_sb[:, mp, :, :], ps[:])
            nc.sync.dma_start(out=p15_d[:], in_=p15_sb[:])

        def do_pair(d, L, slot, blocks, fold=True, flush_here=True):
            exq = expool.tile([128, 4, 2, BAND], fp8e5, tag="exq")
            if len(blocks) == 2:
                for mm in range(4):
                    ps = pspool.tile([128, 2, BAND], fp32, tag="ps", name="ps")
                    for h, s_ in enumerate(blocks):
                        mm_chain(ps[:, h, :], L, s_, mm)
                    if mm == 1 and flush_here:
                        flush_cs()
                    # fused exp + row-sum over both blocks' 1024 cols
                    nc.scalar.activation(
                        out=exq[:, mm, :, :], in_=ps[:], func=AF.Exp, bias=0.0,
                        scale=MM_SCALE, accum_out=rowacc[:, d, mm, slot : slot + 1],
                    )
            else:
                for mp in range(2):
                    ps = pspool.tile([128, 2, BAND], fp32, tag="ps", name="ps")
                    for h in range(2):
                        mm_chain(ps[:, h, :], L, blocks[0], 2 * mp + h)
                    if mp == 0:
                        flush_cs()
                    for h in range(2):
                        mm = 2 * mp + h
                        nc.scalar.activation(
                            out=exq[:, mm, 0, :], in_=ps[:, h, :], func=AF.Exp, bias=0.0,
                            scale=MM_SCALE, accum_out=rowacc[:, d, mm, slot : slot + 1],
                        )
            if fold:
                for h in range(len(blocks)):
                    exf = exfpool.tile([128, 2, BAND], fp8e5, tag="exf", name="exf")
                    nc.gpsimd.tensor_add(
                        exf[:], exq[:, 0:2, h, :], exq[:, 2:4, h, :]
                    )
                    pending_cs.append(lambda exf=exf: emit_cs_folded(exf))
            elif len(blocks) == 2:
                for h in range(2):
                    pending_cs.append(lambda exq=exq, h=h: emit_cs(exq, h))
            else:
                pending_cs.append(lambda exq=exq: emit_cs_half(exq, 0))
                pending_cs.append(lambda exq=exq: emit_cs_half(exq, 1))

        do_diag(0, 0)
        do_pair(0, 0, 0, [1, 2])
        do_pair(0, 0, 1, [3, 4])
        do_pair(0, 0, 2, [5, 6])
        do_pair(0, 0, 3, [7, 8])
        # diag0's exp strip can fly as soon as its last exp lands
        nc.sync.dma_start(out=exdiag_d[:, 0:1, :], in_=exdiag[:, 0:1, :])
        do_single_raw(8, 15)
        do_pair(1, 8, 0, [9, 10])
        do_pair(1, 8, 1, [11, 12])
        do_pair(1, 8, 2, [13, 14], fold=False)
        nc.sync.dma_start(out=rowacc_d[:], in_=rowacc[:])
        cs_state["tail"] = True
        do_diag(1, 8, last=True)
        flush_cs()
        for fn in pending_copies:
            fn()
        pending_copies.clear()
        nc.sync.dma_start(out=exdiag_d[:, 1:2, :], in_=exdiag[:, 1:2, :])
        nc.sync.dma_start(out=colsum_d[:], in_=cs_sb[0:1, :, :])

    nc.compile()
    return nc


def _make_in_maps(z1: np.ndarray, z2: np.ndarray):
    z = np.concatenate([z1, z2], axis=0)          # [8192, 1024] f32
    zt = (z.T * FP8_SCALE).astype(np.float32)     # [1024, 8192]
    # [NB, 128, KT, BAND] band-major fp8 tiles: band, k-within-tile, k-tile, col
    zb = np.ascontiguousarray(
        zt.reshape(KT, 128, NB, BAND).transpose(2, 1, 0, 3)
    ).astype(ml_dtypes.float8_e4m3)
    return [
        {"z": np.ascontiguousarray(zb[[(c + s) % NB for s in range(NB)]])}
        for c in range(N_CORES)
    ]


def kernel(z1: np.ndarray, z2: np.ndarray) -> np.ndarray:
    assert z1.shape == (N, D) and z2.shape == (N, D)
    in_maps = _make_in_maps(z1, z2)

    if "nc" not in _CACHE:
        _CACHE["nc"] = _build_graph()
    res = run_bass_kernel_spmd(_CACHE["nc"], in_maps, core_ids=list(range(N_CORES)))

    S = np.zeros(ROWS, np.float64)
    for c in range(N_CORES):
        r = res.results[c]
        ra = np.asarray(r["rowacc"], dtype=np.float64)     # [128, 2, 4, 4]
        cs = np.asarray(r["colsum"], dtype=np.float64)[0]  # [15, 512]
        ed = np.asarray(r["exdiag"], dtype=np.float64)     # [128, 2, 512]
        pc = np.asarray(r["psc"], dtype=np.float64)        # [128, 2, 768]
        for d, L in ((0, 0), (1, 8)):
            b = (c + L) % NB
            base = BAND * b
            vals = ra[:, d, :, :].sum(axis=2)                 # [128 p, 4 m]
            S[base : base + BAND] += vals.T.reshape(BAND)
            # diag block, upper-triangle chains: chains 0,1 as on-chip exp
            # tiles, chains 2,3 as raw sim/(64T) PSUM (mask+exp here).
            # Upper row sums + (by symmetry) lower row sums = column sums
            # of the strictly-upper regions.
            for mm in range(4):
                w = BAND - 128 * mm
                if mm == 0:
                    E = ed[:, d, 0:w]                         # rows 0..128
                else:
                    off = (0, 0, 384, 640)[mm]
                    E = np.exp(pc[:, d, off : off + w] * MM_SCALE)
                    np.fill_diagonal(E, 0.0)
                S[base + 128 * mm : base + 128 * (mm + 1)] += E.sum(axis=1)
                if w > 128:
                    S[base + 128 * (mm + 1) : base + BAND] += E[:, 128:w].sum(axis=0)
        for ci, s in enumerate(list(range(1, 9)) + list(range(9, 15))):
            b = (c + s) % NB
            S[BAND * b : BAND * (b + 1)] += cs[ci, :]
        # block (L=8, s=15): raw sim/(64T); exp here, rows + cols
        p15 = np.asarray(r["p15"], dtype=np.float64)       # [128, 2, 2, 512]
        E15 = np.exp(p15 * MM_SCALE)
        b8, b15 = (c + 8) % NB, (c + 15) % NB
        rows15 = E15.sum(axis=3).transpose(1, 2, 0).reshape(BAND)
        S[BAND * b8 : BAND * (b8 + 1)] += rows15
        S[BAND * b15 : BAND * (b15 + 1)] += E15.sum(axis=(0, 1, 2))
    pos = (z1.astype(np.float64) * z2.astype(np.float64)).sum(axis=1) / TEMP
    loss = np.log(S).mean() - pos.mean()
    return np.asarray(loss, dtype=np.float32)
